# revision 10
# baseline (speedup 1.0000x reference)
"""Multi-head attention Trainium2 kernel (8 NeuronCores, Bass/Tile).

Sharding: core c -> (batch b = c//2, head-group hg = c%2). Each core computes
attention for 8 of the 16 heads of one batch element plus its partial
out-projection; the host sums the two head-group partials per batch.

Per-core layouts (host pre-transposes inputs; contraction dims on partitions):
  xT  [E=1024, S=2048]      x[b].T
  wqT/wkT/wvT [1024, 512]   W[hg_rows].T
  woT [512, 1024]           Wo[:, hg_cols].T
  sel2 [2, 128]             0/1 selector for softmax-denominator replication

On-chip pipeline (bf16 matmul operands, fp32 PSUM accumulation/softmax):
  QT = wqT.T-tiles @ xT   [512, 2048] (head-major, transposed)
  KT likewise; V natural [2048, 512]
  scoresT[t,s] = KT_h.T-tile @ QT_h   (K=64; two heads row-packed -> concurrent)
  expT = exp(scoresT/8) on ScalarE    ([128,1024] psum->sbuf per t-tile)
  outT = V_h.T @ expT                 (col-packed pair -> concurrent)
  Z    = ones.T @ expT                (M=1, 4-way col-tiled -> rides in gaps)
  outT_norm = outT * rep(1/Z)         (per-pair K=2 selector matmul + DVE mult)
  out = outT_norm.T-tiles @ woT       [2048, 1024] partial, stored bf16

Schedule: dual-pair groups as the packing-optimal inner loop (4 zz sums
pack across quadrant columns; score/av pairs run concurrently). Startup is
slimmed: PE p-state warmup + Exp-table preload run during the x DMA, and
only K(m0,m1), Q(m0,m1,chunk0) and V(t0-3) are projected upfront; all other
projections drip as whole pipelined chains (front-loaded, one per step)
with ensure() force-emitting any prereq before its consumer enters the
in-order PE stream. Softmax normalization is per-pair (K=2 selector,
[2,512] reciprocal staged via DMA) so chunk out-projections spread out;
the final chunk's out-projection alternates the freed av banks. Output is
written in bf16 (halves the output DMA); the host accumulates the two
head-group partials in fp32.
"""

import os
import sys
import types

import numpy as np

B, S, E, H = 4, 2048, 1024, 16
DK = E // H  # 64
HG = H // 2  # heads per core = 8
DG = HG * DK  # 512 projected dims per core
NCORES = 8

TRACE = bool(os.environ.get("TRN_KERNEL_TRACE"))
# matmul-operand dtype: bf16 single-pass PE (fp32 PSUM accumulation) vs
# fp32 operands (PE double-pumps each matmul -> ~2x slower)
MM_DTYPE = os.environ.get("TRN_MM_DTYPE", "bf16")
LAST_EXEC_TIME_NS = None

_cache = {}

def _make_sel():
    # zbuf row for head (2k + p//64): even heads -> row k, odd heads -> row 4+k
    sel = np.zeros((HG, 512), dtype=np.float32)
    for k in range(4):
        for p in range(128):
            r = k if p < 64 else 4 + k
            sel[r, k * 128 + p] = 1.0
    return sel


def _env_setup():
    import antenv

    if "antenv.axon_hooks" not in sys.modules:
        mod = types.ModuleType("antenv.axon_hooks")
        mod._hook = None
        mod.set_axon_ntff_profile_hook = lambda h: setattr(mod, "_hook", h)
        mod.get_axon_ntff_profile_hook = lambda: mod._hook
        sys.modules["antenv.axon_hooks"] = mod
        antenv.axon_hooks = mod
        try:
            from trn_agent_boot.trn_boot import _ntff_profile_via_ctypes

            mod.set_axon_ntff_profile_hook(
                _ntff_profile_via_ctypes("/opt/axon/libaxon_pjrt.so")
            )
        except Exception:
            pass

    import concourse.bass_utils as bass_utils

    bass_utils.upload_artifacts = lambda tmpdir: tmpdir

    import concourse.tile as tile
    from concourse import mybir
    from concourse.vector_clock import ScopedClock

    if getattr(tile.TileContext, "_wait_split_patched", False):
        return

    MAX_WAITS = 1  # walrus on this image rejects >1 sync wait per instruction

    def _drain_and_barrier_split(self, tick_clock, wait_clock):
        probe = self.nc.sync.drain()
        wait_clock.add_sem_waits(
            probe.ins, ScopedClock({None: tick_clock.global_clock})
        )
        waits = list(probe.ins.sync_info.on_wait)
        if len(waits) > MAX_WAITS:
            num2h = {h.num: h for h in self.sems.allocated().values()}
            probe.ins.sync_info.on_wait = []
            for w in waits:
                self.nc.sync.wait_ge(num2h[w.id], w.wait_value)
            self.nc.sync.drain()
        self.nc.all_engine_barrier()
        popped = self.nc._tile_sem_poison_stack.pop()
        assert popped is self._sem_poison
        self.nc.clear_and_free_semaphores(list(self.sems.allocated().values()))
        self.nc.all_engine_barrier()

    _orig_commit = tile.TileContext._commit_instruction
    _ctr = [0]

    def _commit_split_waits(self, inst, lazy_reg_writes=True):
        si = inst.sync_info
        if (
            si is not None
            and len(si.on_wait) > MAX_WAITS
            and inst.engine != mybir.EngineType.Unassigned
        ):
            waits = list(si.on_wait)
            keep, hoist = waits[:MAX_WAITS], waits[MAX_WAITS:]
            for i in range(0, len(hoist), MAX_WAITS):
                _ctr[0] += 1
                nop = mybir.InstNoOp(name=f"waitnop-{_ctr[0]}", ins=[], outs=[])
                nop.engine = inst.engine
                nop.sync_info = mybir.SyncInfo(
                    on_wait=hoist[i : i + MAX_WAITS], on_update=[]
                )
                self.nc.register_instruction(nop, overwrite=True)
                _orig_commit(self, nop, lazy_reg_writes=False)
            inst.sync_info = mybir.SyncInfo(on_wait=keep, on_update=list(si.on_update))
        return _orig_commit(self, inst, lazy_reg_writes=lazy_reg_writes)

    tile.TileContext._drain_and_barrier = _drain_and_barrier_split
    tile.TileContext._commit_instruction = _commit_split_waits
    tile.TileContext._wait_split_patched = True

    # use the full usable SBUF on trn2 (default constant is stale)
    import concourse.tile_utils as tile_utils

    tile_utils.max_sbuf_usage = 206 * 1024


def _build_nc():
    import contextlib

    import concourse.bass as bass
    import concourse.tile as tile
    from concourse import mybir

    F32 = mybir.dt.float32
    CDT = mybir.dt.bfloat16 if MM_DTYPE == "bf16" else mybir.dt.float32
    PS = bass.MemorySpace.PSUM
    AF = mybir.ActivationFunctionType

    nc = bass.Bass()
    xT_d = nc.dram_tensor("xT", [E, S], CDT, kind="ExternalInput")
    wqT_d = nc.dram_tensor("wqT", [E, DG], CDT, kind="ExternalInput")
    wkT_d = nc.dram_tensor("wkT", [E, DG], CDT, kind="ExternalInput")
    wvT_d = nc.dram_tensor("wvT", [E, DG], CDT, kind="ExternalInput")
    woT_d = nc.dram_tensor("woT", [DG, E], CDT, kind="ExternalInput")
    sel_d = nc.dram_tensor("sel", [HG, 512], CDT, kind="ExternalInput")
    out_d = nc.dram_tensor("out", [S, E], CDT, kind="ExternalOutput")

    NE = E // 128  # 8 e-tiles
    NT = S // 128  # 16 t/s-tiles
    NNC = S // 512  # 4 s-chunks
    NM = DG // 128  # 4 head-pair tiles
    DLY = 2  # av/sums trail scores/exp by DLY steps

    with tile.TileContext(nc) as tc:
        st = contextlib.ExitStack()
        with st:
            pp = st.enter_context(tc.tile_pool(name="persist", bufs=1))
            stg = st.enter_context(tc.tile_pool(name="stage", bufs=6))
            expp = st.enter_context(tc.tile_pool(name="expp", bufs=8))
            outp = st.enter_context(tc.tile_pool(name="outp", bufs=4))
            zsp = st.enter_context(tc.tile_pool(name="zsp", bufs=2))

            QT = pp.tile([128, NM * S], CDT, tag="QT")  # [128, 8192]
            KT = pp.tile([128, NM * S], CDT, tag="KT")
            Vsb = pp.tile([128, NT * DG], CDT, tag="V")  # [128, 8192]
            onorm = pp.tile([128, NM * S], CDT, tag="onorm")
            woT = pp.tile([128, NM * E], CDT, tag="woT")  # [128, 4096]
            ones = pp.tile([128, 1], CDT, tag="ones")
            nc.gpsimd.memset(ones[:], 1.0)
            sel_sb = pp.tile([HG, 512], CDT, tag="sel")
            zbuf = pp.tile([HG, S], F32, tag="zbuf")
            zrecc = pp.tile([HG, S], CDT, tag="zrecc")
            zstage = pp.tile([128, NM * 512], F32, tag="zstage")
            zrec32 = pp.tile([HG, 512], F32, tag="zrec32")
            warm = pp.tile([128, 256], CDT, tag="warm")
            nc.gpsimd.memset(warm[:], 0.001)
            wscr = pp.tile([128, 16], CDT, tag="wscr")

            xT = pp.tile([128, NE * S], CDT, tag="xT")  # [128, 16384]
            wq = pp.tile([128, NE * DG], CDT, tag="wq")
            wk = pp.tile([128, NE * DG], CDT, tag="wk")
            wv = pp.tile([128, NE * DG], CDT, tag="wv")

            # DMA order: per e-tile, x then the m0/m1 halves of Wk/Wq (the
            # upfront chains track per-tile arrivals); then Wv, woT, the
            # m2/m3 halves, sel2.
            for j in range(NE):
                nc.sync.dma_start(
                    xT[:, j * S : (j + 1) * S], xT_d[j * 128 : (j + 1) * 128, :]
                )
                nc.sync.dma_start(
                    wk[:, j * DG : j * DG + 256],
                    wkT_d[j * 128 : (j + 1) * 128, 0:256],
                )
                nc.sync.dma_start(
                    wq[:, j * DG : j * DG + 256],
                    wqT_d[j * 128 : (j + 1) * 128, 0:256],
                )
            for j in range(NE):
                nc.sync.dma_start(
                    wv[:, j * DG : (j + 1) * DG],
                    wvT_d[j * 128 : (j + 1) * 128, :],
                )
            for k in range(NM):
                nc.sync.dma_start(
                    woT[:, k * E : (k + 1) * E], woT_d[k * 128 : (k + 1) * 128, :]
                )
            for j in range(NE):
                nc.sync.dma_start(
                    wk[:, j * DG + 256 : (j + 1) * DG],
                    wkT_d[j * 128 : (j + 1) * 128, 256:512],
                )
                nc.sync.dma_start(
                    wq[:, j * DG + 256 : (j + 1) * DG],
                    wqT_d[j * 128 : (j + 1) * 128, 256:512],
                )
            nc.sync.dma_start(sel_sb[:], sel_d[:])

            # -------- upfront: warm PE p-state + preload Exp table during
            # the x DMA, then K(m0,m1) all chunks + Q(m0,m1) chunk 0.
            with tc.tile_pool(name="projps", bufs=3, space=PS) as proj_ps:
                wps = proj_ps.tile([128, 256], F32, tag="pj", name="warmps")
                for r in range(56):
                    nc.tensor.matmul(
                        wps[:], warm[:, 0:128], warm[:],
                        start=(r == 0), stop=(r == 55),
                    )
                nc.scalar.activation(wscr[:], warm[:, 0:16], AF.Exp, scale=0.125)

                def up_qk(w_sb, dst, m, n):
                    acc = proj_ps.tile([128, 512], F32, tag="pj")
                    for j in range(NE):
                        nc.tensor.matmul(
                            acc[:],
                            w_sb[:, j * DG + m * 128 : j * DG + (m + 1) * 128],
                            xT[:, j * S + n * 512 : j * S + (n + 1) * 512],
                            start=(j == 0),
                            stop=(j == NE - 1),
                        )
                    nc.vector.tensor_copy(
                        dst[:, m * S + n * 512 : m * S + (n + 1) * 512], acc[:]
                    )

                for m in (0, 1):
                    for n in range(NNC):
                        up_qk(wk, KT, m, n)
                up_qk(wq, QT, 0, 0)
                up_qk(wq, QT, 1, 0)

            sc_ps = st.enter_context(tc.tile_pool(name="scpsum", bufs=2, space=PS))
            av_ps = st.enter_context(tc.tile_pool(name="avpsum", bufs=2, space=PS))
            z_ps = st.enter_context(tc.tile_pool(name="zpsum", bufs=1, space=PS))
            ms_ps = st.enter_context(tc.tile_pool(name="miscpsum", bufs=1, space=PS))

            # -------- drip queues. qhigh: per-pair normalize (atomic ms
            # closures). qlow: whole projection / out-projection chains
            # (atomic; pipeline internally at full PE issue rate).
            qlow = []
            emitted = set()

            def v_chain(i):
                def fn(i=i):
                    acc = ms_ps.tile([128, 512], F32, tag="ms", name=f"vch{i}")
                    for j in range(NE):
                        nc.tensor.matmul(
                            acc[:],
                            xT[:, j * S + i * 128 : j * S + (i + 1) * 128],
                            wv[:, j * DG : (j + 1) * DG],
                            start=(j == 0),
                            stop=(j == NE - 1),
                        )
                    nc.vector.tensor_copy(Vsb[:, i * DG : (i + 1) * DG], acc[:])
                    emitted.add(("v", i))

                qlow.append(fn)

            def qk_chain(key, w_sb, dst, m, n):
                def fn(key=key, m=m, n=n, w_sb=w_sb, dst=dst):
                    acc = ms_ps.tile(
                        [128, 512], F32, tag="ms", name=f"qk{m}_{n}"
                    )
                    for j in range(NE):
                        nc.tensor.matmul(
                            acc[:],
                            w_sb[:, j * DG + m * 128 : j * DG + (m + 1) * 128],
                            xT[:, j * S + n * 512 : j * S + (n + 1) * 512],
                            start=(j == 0),
                            stop=(j == NE - 1),
                        )
                    nc.vector.tensor_copy(
                        dst[:, m * S + n * 512 : m * S + (n + 1) * 512], acc[:]
                    )
                    emitted.add(key)

                qlow.append(fn)

            def pop_one():
                if qlow:
                    qlow.pop(0)()

            def ensure(key):
                while key not in emitted:
                    assert qlow, f"prereq {key} unsatisfiable"
                    pop_one()

            for m in (0, 1):
                emitted.add(("k", m))
                emitted.add(("q", m, 0))

            # group order (chunk n, lead pair hpp): as baseline — spreads
            # chunk completions across the run
            GROUPS = [
                (0, 0), (1, 0), (2, 0), (0, 2), (3, 0), (1, 2), (2, 2), (3, 2)
            ]

            # drip registration in prereq order: V tails first (group 0
            # consumes V_t from step t+2), then per-group K/Q chains
            for i in range(4, NT):
                v_chain(i)
            seen_k = {0, 1}
            seen_q = {(0, 0), (1, 0)}
            for n, hpp in GROUPS[1:]:
                for m in (hpp, hpp + 1):
                    if m not in seen_k:
                        seen_k.add(m)
                        for nn in range(NNC):
                            qk_chain(
                                ("k", m) if nn == NNC - 1 else ("kp", m, nn),
                                wk, KT, m, nn,
                            )
                    if (m, n) not in seen_q:
                        seen_q.add((m, n))
                        qk_chain(("q", m, n), wq, QT, m, n)

            # -------- boundary: stage uo + Z per pair inline (DVE/DMA),
            # per-chunk reciprocal; normalize + chunk out-projection drip
            # through the same FIFO behind any remaining projection chains.
            uos = {}
            ndone = {n: 0 for n in range(NNC)}

            def boundary(n, hpp, av_a, av_b, zz, is_last):
                for hp, av, zrow in ((hpp, av_a, 0), (hpp + 1, av_b, 64)):
                    uo = stg.tile([128, 512], F32, tag="uo", name=f"uo{n}{hp}")
                    nc.vector.tensor_copy(uo[:], av[:])
                    uos[(n, hp)] = uo
                    nc.vector.tensor_copy(
                        zstage[0:1, hp * 512 : (hp + 1) * 512],
                        zz[zrow : zrow + 1, :],
                    )
                    nc.vector.tensor_copy(
                        zstage[32:33, hp * 512 : (hp + 1) * 512],
                        zz[zrow + 32 : zrow + 33, :],
                    )
                    nc.sync.dma_start(
                        zbuf[hp : hp + 1, n * 512 : (n + 1) * 512],
                        zstage[0:1, hp * 512 : (hp + 1) * 512],
                    )
                    nc.sync.dma_start(
                        zbuf[4 + hp : 5 + hp, n * 512 : (n + 1) * 512],
                        zstage[32:33, hp * 512 : (hp + 1) * 512],
                    )
                ndone[n] += 1
                if ndone[n] < 2:
                    return
                with nc.allow_low_precision(reason="1/Z replicated in bf16"):
                    nc.vector.reciprocal(
                        zrecc[:, n * 512 : (n + 1) * 512],
                        zbuf[:, n * 512 : (n + 1) * 512],
                    )
                for k in range(NM):
                    def norm_k(k=k, n=n, is_last=is_last):
                        if is_last:
                            rep = z_ps.tile(
                                [128, 512], F32, tag="zz", name=f"rp{n}{k}"
                            )
                        else:
                            rep = ms_ps.tile(
                                [128, 512], F32, tag="ms", name=f"rp{n}{k}"
                            )
                        nc.tensor.matmul(
                            rep[:],
                            sel_sb[:, k * 128 : (k + 1) * 128],
                            zrecc[:, n * 512 : (n + 1) * 512],
                        )
                        uo_t = uos.pop((n, k))
                        nc.vector.tensor_tensor(
                            onorm[:, k * S + n * 512 : k * S + (n + 1) * 512],
                            uo_t[:],
                            rep[:],
                            mybir.AluOpType.mult,
                        )

                    qlow.append(norm_k)
                osbs = {}
                for i in range(4 * n, 4 * n + 4):
                    def o_pre(i=i):
                        osbs[i] = outp.tile(
                            [128, E], CDT, tag="osb", name=f"osb{i}"
                        )

                    qlow.append(o_pre)
                    for eh in (0, 1):
                        def o_mm(i=i, eh=eh, is_last=is_last):
                            if is_last:
                                ps = av_ps.tile(
                                    [128, 512], F32, tag="av",
                                    name=f"olp{i}{eh}",
                                )
                            else:
                                ps = ms_ps.tile(
                                    [128, 512], F32, tag="ms",
                                    name=f"op{i}{eh}",
                                )
                            for k in range(NM):
                                nc.tensor.matmul(
                                    ps[:],
                                    onorm[:, k * S + i * 128 : k * S + (i + 1) * 128],
                                    woT[:, k * E + eh * 512 : k * E + (eh + 1) * 512],
                                    start=(k == 0),
                                    stop=(k == NM - 1),
                                )
                            nc.vector.tensor_copy(
                                osbs[i][:, eh * 512 : (eh + 1) * 512], ps[:]
                            )
                            nc.sync.dma_start(
                                out_d[
                                    i * 128 : (i + 1) * 128,
                                    eh * 512 : (eh + 1) * 512,
                                ],
                                osbs[i][:, eh * 512 : (eh + 1) * 512],
                            )

                        qlow.append(o_mm)

            # -------- attention
            def score_step(hp, n, t):
                sc = sc_ps.tile([128, 1024], F32, tag="sc")
                nc.tensor.matmul(
                    sc[:, 0:512],
                    KT[0:64, hp * S + t * 128 : hp * S + (t + 1) * 128],
                    QT[0:64, hp * S + n * 512 : hp * S + (n + 1) * 512],
                )
                nc.tensor.matmul(
                    sc[:, 512:1024],
                    KT[64:128, hp * S + t * 128 : hp * S + (t + 1) * 128],
                    QT[64:128, hp * S + n * 512 : hp * S + (n + 1) * 512],
                )
                ex = expp.tile([128, 1024], CDT, tag="ex")
                nc.scalar.activation(ex[:], sc[:], AF.Exp, scale=0.125)
                return ex

            def av_pair(hp, t, av, ex):
                voff = t * DG
                nc.tensor.matmul(
                    av[0:64, :],
                    Vsb[:, voff + (2 * hp) * DK : voff + (2 * hp) * DK + DK],
                    ex[:, 0:512],
                    start=(t == 0),
                    stop=(t == NT - 1),
                    tile_position=(0, 0),
                    skip_group_check=True,
                )
                nc.tensor.matmul(
                    av[64:128, :],
                    Vsb[:, voff + (2 * hp + 1) * DK : voff + (2 * hp + 1) * DK + DK],
                    ex[:, 512:1024],
                    start=(t == 0),
                    stop=(t == NT - 1),
                    tile_position=(0, 64),
                    skip_group_check=True,
                )

            def sum_pair(t, zz, zrow, ex):
                nc.tensor.matmul(
                    zz[zrow : zrow + 1, :],
                    ones[:, 0:1],
                    ex[:, 0:512],
                    start=(t == 0),
                    stop=(t == NT - 1),
                    tile_position=(0, zrow),
                    skip_group_check=True,
                )
                nc.tensor.matmul(
                    zz[zrow + 32 : zrow + 33, :],
                    ones[:, 0:1],
                    ex[:, 512:1024],
                    start=(t == 0),
                    stop=(t == NT - 1),
                    tile_position=(0, zrow + 32),
                    skip_group_check=True,
                )

            first = True
            for gi, (n, hpp) in enumerate(GROUPS):
                ensure(("k", hpp))
                ensure(("k", hpp + 1))
                ensure(("q", hpp, n))
                ensure(("q", hpp + 1, n))
                av_a = av_ps.tile([128, 512], F32, tag="av")
                av_b = av_ps.tile([128, 512], F32, tag="av")
                zz = z_ps.tile([128, 512], F32, tag="zz")
                pend = []
                if first:
                    # seed the exp pipeline, then project V t0-3 while the
                    # first two exps run
                    for t0 in range(2):
                        pend.append(
                            (t0, score_step(hpp, n, t0), score_step(hpp + 1, n, t0))
                        )
                    for i in range(4):
                        acc = ms_ps.tile([128, 512], F32, tag="ms", name=f"v0{i}")
                        for j in range(NE):
                            nc.tensor.matmul(
                                acc[:],
                                xT[:, j * S + i * 128 : j * S + (i + 1) * 128],
                                wv[:, j * DG : (j + 1) * DG],
                                start=(j == 0),
                                stop=(j == NE - 1),
                            )
                        nc.vector.tensor_copy(
                            Vsb[:, i * DG : (i + 1) * DG], acc[:]
                        )
                        emitted.add(("v", i))
                for t in range(2 if first else 0, NT + DLY):
                    if t < NT:
                        pend.append(
                            (t, score_step(hpp, n, t), score_step(hpp + 1, n, t))
                        )
                    if len(pend) > DLY or t >= NT:
                        pt, pa, pb = pend.pop(0)
                        ensure(("v", pt))
                        av_pair(hpp, pt, av_a, pa)
                        av_pair(hpp + 1, pt, av_b, pb)
                        sum_pair(pt, zz, 0, pa)
                        sum_pair(pt, zz, 64, pb)
                    if t >= 3 and (gi < len(GROUPS) - 1 or t % 3 == 0):
                        pop_one()
                first = False
                boundary(n, hpp, av_a, av_b, zz, is_last=(gi == len(GROUPS) - 1))

            while qlow:
                pop_one()

    return nc


def kernel(x, Wq, Wk, Wv, Wo):
    global LAST_EXEC_TIME_NS
    _env_setup()
    from concourse.bass_utils import run_bass_kernel_spmd

    x = np.asarray(x, dtype=np.float32)
    Wq = np.asarray(Wq, dtype=np.float32)
    Wk = np.asarray(Wk, dtype=np.float32)
    Wv = np.asarray(Wv, dtype=np.float32)
    Wo = np.asarray(Wo, dtype=np.float32)

    if "nc" not in _cache:
        _cache["nc"] = _build_nc()
    nc = _cache["nc"]

    if MM_DTYPE == "bf16":
        import ml_dtypes

        cdt = ml_dtypes.bfloat16
    else:
        cdt = np.float32

    in_maps = []
    for c in range(NCORES):
        b, hg = c // 2, c % 2
        r = slice(hg * DG, (hg + 1) * DG)
        in_maps.append(
            {
                "xT": np.ascontiguousarray(x[b].T).astype(cdt),
                "wqT": np.ascontiguousarray(Wq[r, :].T).astype(cdt),
                "wkT": np.ascontiguousarray(Wk[r, :].T).astype(cdt),
                "wvT": np.ascontiguousarray(Wv[r, :].T).astype(cdt),
                "woT": np.ascontiguousarray(Wo[:, r].T).astype(cdt),
                "sel": _make_sel().astype(cdt),
            }
        )

    res = run_bass_kernel_spmd(
        nc, in_maps, core_ids=list(range(NCORES)), trace=TRACE
    )
    if TRACE:
        LAST_EXEC_TIME_NS = res.exec_time_ns

    out = np.empty((B, S, E), dtype=np.float32)
    for b in range(B):
        out[b] = np.asarray(res.results[2 * b]["out"], dtype=np.float32) + np.asarray(
            res.results[2 * b + 1]["out"], dtype=np.float32
        )
    return out
